# revision 1
# baseline (speedup 1.0000x reference)
"""Trainium2 Bass kernel for nn_NodeModel (GNN message passing).

Reference computation:
    h   = relu(concat(x[row], edge_attr) @ W1 + b1) @ W2 + b2     # edge MLP
    agg = scatter_mean(h, col, N)                                  # per-dest mean
    out = relu(concat(x, agg) @ W3 + b3) @ W4 + b4                 # node MLP

Distribution strategy (8 cores, no collectives needed):
  - Sort edges by destination node; split destination nodes into 8
    block-aligned, edge-balanced shards.  Each core owns one node shard and
    ALL edges targeting it, so per-node sums are complete locally.
  - x is replicated; each core gathers x[row] for its edges with indirect
    DMA on-device.
  - Edge MLP runs with weights stationary and activations kept transposed
    [feat, edge]; h2 rows are staged to DRAM.
  - Scatter-mean per 128-node block: indirect-gather the block's h2 rows,
    build a one-hot selection matrix with is_equal against an iota, and
    matmul-accumulate S^T @ h2 in PSUM; scale by 1/count.
  - Node MLP on the local shard; outputs are concatenated on host.

All matmuls run in float32r (TF32-like, full PE rate); accumulation fp32.
"""

import math
import sys
from contextlib import ExitStack

sys.path.insert(0, "/opt/trn_rl_repo")

import numpy as np

import concourse.bass as bass
import concourse.tile as tile
from concourse import bacc, mybir
from concourse.bass_utils import run_bass_kernel_spmd

NCORES = 8
P = 128
FN = 512    # node feature dim
FE = 128    # edge feature dim
HID = 1280  # edge-MLP hidden/output dim
F32 = mybir.dt.float32
F32R = mybir.dt.float32r
I32 = mybir.dt.int32
RELU = mybir.ActivationFunctionType.Relu

_prog_cache = {}


def _build(EC, NB, KB, NX):
    """Build the SPMD program for one core.

    EC: edge chunks (128 edges each) per core, multiple of 4.
    NB: node blocks (128 nodes each) per core, multiple of 4.
    KB: max edge chunks per node block (scatter schedule width).
    NX: number of rows of the replicated x (gather source).
    """
    EP = EC * P
    NBP = NB * P
    SC = EC // 4   # superchunks of 512 edges
    NSB = NB // 4  # superblocks of 512 nodes

    nc = bacc.Bacc("TRN2", target_bir_lowering=False, debug=False,
                   num_devices=NCORES)

    x_d = nc.dram_tensor("x", [NX, FN], F32R, kind="ExternalInput")
    rows_d = nc.dram_tensor("rows", [P, EC], I32, kind="ExternalInput")
    eaT_d = nc.dram_tensor("eaT", [FE, EP], F32R, kind="ExternalInput")
    W1_d = nc.dram_tensor("W1", [FN + FE, HID], F32R, kind="ExternalInput")
    W2_d = nc.dram_tensor("W2", [HID, HID], F32R, kind="ExternalInput")
    W3_d = nc.dram_tensor("W3", [FN + HID, FN + FE], F32R, kind="ExternalInput")
    W4_d = nc.dram_tensor("W4", [FN + FE, FN], F32R, kind="ExternalInput")
    b1_d = nc.dram_tensor("b1", [P, HID // P], F32, kind="ExternalInput")
    b2_d = nc.dram_tensor("b2", [P, HID // P], F32, kind="ExternalInput")
    b3_d = nc.dram_tensor("b3", [P, (FN + FE) // P], F32, kind="ExternalInput")
    b4_d = nc.dram_tensor("b4", [P, FN // P], F32, kind="ExternalInput")
    gid_d = nc.dram_tensor("gid", [P, NB * KB], I32, kind="ExternalInput")
    colb_d = nc.dram_tensor("colb", [P, NB * KB], F32, kind="ExternalInput")
    invc_d = nc.dram_tensor("invc", [P, NB], F32, kind="ExternalInput")
    xsT_d = nc.dram_tensor("xsT", [FN, NBP], F32R, kind="ExternalInput")
    iota_d = nc.dram_tensor("iota", [P, P], F32, kind="ExternalInput")
    ident_d = nc.dram_tensor("ident", [P, P], F32R, kind="ExternalInput")
    out_d = nc.dram_tensor("out", [NBP, FN], F32, kind="ExternalOutput")
    h2_d = nc.dram_tensor("h2buf", [EP, HID], F32R)  # internal staging

    with tile.TileContext(nc) as tc, ExitStack() as ctx:
        cpool = ctx.enter_context(tc.tile_pool(name="const", bufs=1))

        identt = cpool.tile([P, P], F32R)
        nc.sync.dma_start(identt[:], ident_d.ap()[:])
        iotat = cpool.tile([P, P], F32)
        nc.sync.dma_start(iotat[:], iota_d.ap()[:])
        b1t = cpool.tile([P, HID // P], F32)
        nc.sync.dma_start(b1t[:], b1_d.ap()[:])
        b2t = cpool.tile([P, HID // P], F32)
        nc.sync.dma_start(b2t[:], b2_d.ap()[:])
        b3t = cpool.tile([P, (FN + FE) // P], F32)
        nc.sync.dma_start(b3t[:], b3_d.ap()[:])
        b4t = cpool.tile([P, FN // P], F32)
        nc.sync.dma_start(b4t[:], b4_d.ap()[:])
        rowst = cpool.tile([P, EC], I32)
        nc.sync.dma_start(rowst[:], rows_d.ap()[:])
        gidt = cpool.tile([P, NB * KB], I32)
        nc.sync.dma_start(gidt[:], gid_d.ap()[:])
        colbt = cpool.tile([P, NB * KB], F32)
        nc.sync.dma_start(colbt[:], colb_d.ap()[:])
        invct = cpool.tile([P, NB], F32)
        nc.sync.dma_start(invct[:], invc_d.ap()[:])

        # ---------------- Phase E: edge MLP ----------------
        # Transposes run in PE transpose-mode, which does NOT count as
        # PE activity for the HAM clock gate: a burst of back-to-back
        # transposes >3.4us re-throttles the PE to 1.2 GHz.  All
        # transposes are therefore interleaved between matmul groups,
        # and gathers are pipelined one superchunk ahead.
        with ExitStack() as ectx:
            wpool = ectx.enter_context(tc.tile_pool(name="wE", bufs=1))
            W1t = wpool.tile([P, 5, HID], F32R)
            W1r = W1_d.ap().rearrange("(ko ki) m -> ki ko m", ki=P)
            for k in range(5):
                nc.sync.dma_start(W1t[:, k, :], W1r[:, k, :])
            W2t = wpool.tile([P, 10, HID], F32R)
            W2r = W2_d.ap().rearrange("(ko ki) m -> ki ko m", ki=P)
            for k in range(10):
                nc.sync.dma_start(W2t[:, k, :], W2r[:, k, :])

            ptp = ectx.enter_context(
                tc.tile_pool(name="ptp", bufs=2, space="PSUM"))
            xgp = ectx.enter_context(tc.tile_pool(name="xg", bufs=2))
            xgTp = ectx.enter_context(tc.tile_pool(name="xgT", bufs=2))
            eap = ectx.enter_context(tc.tile_pool(name="ea", bufs=2))
            h1p = ectx.enter_context(tc.tile_pool(name="h1T", bufs=1))
            h2Tp = ectx.enter_context(tc.tile_pool(name="h2T", bufs=1))
            h2op = ectx.enter_context(tc.tile_pool(name="h2o", bufs=4))
            mmp = ectx.enter_context(
                tc.tile_pool(name="mmE", bufs=4, space="PSUM"))

            def issue_gather(sc):
                xgt = xgp.tile([P, 4, FN], F32R)
                for k in range(4):
                    nc.gpsimd.indirect_dma_start(
                        out=xgt[:, k, :], out_offset=None, in_=x_d.ap()[:],
                        in_offset=bass.IndirectOffsetOnAxis(
                            ap=rowst[:, sc * 4 + k:sc * 4 + k + 1], axis=0))
                eat = eap.tile([P, 512], F32R)
                nc.sync.dma_start(
                    eat[:], eaT_d.ap()[:, sc * 512:(sc + 1) * 512])
                return xgt, eat

            def entry_T(xgt, xgTt, f, k):
                pt = ptp.tile([P, P], F32R)
                nc.tensor.transpose(
                    pt[:], xgt[:, k, f * P:(f + 1) * P], identt[:])
                nc.vector.tensor_copy(xgTt[:, f, k * P:(k + 1) * P], pt[:])

            # prologue: superchunk 0 input + its entry transposes
            xg_cur, ea_cur = issue_gather(0)
            xgT_cur = xgTp.tile([P, 4, 512], F32R)
            for f in range(4):
                for k in range(4):
                    entry_T(xg_cur, xgT_cur, f, k)

            for sc in range(SC):
                if sc + 1 < SC:
                    xg_next, ea_next = issue_gather(sc + 1)
                    xgT_next = xgTp.tile([P, 4, 512], F32R)
                else:
                    xg_next = ea_next = xgT_next = None

                h1Tt = h1p.tile([P, 10, 512], F32R)
                for of in range(10):
                    ps = mmp.tile([P, 512], F32)
                    for k in range(5):
                        rhs = xgT_cur[:, k, :] if k < 4 else ea_cur[:]
                        nc.tensor.matmul(
                            ps[:], W1t[:, k, of * P:(of + 1) * P], rhs,
                            start=(k == 0), stop=(k == 4))
                    nc.scalar.activation(h1Tt[:, of, :], ps[:], RELU,
                                         bias=b1t[:, of:of + 1])
                h2Tt = h2Tp.tile([P, 10, 512], F32R)
                h2ot = [h2op.tile([P, HID], F32R, name=f"h2o_{sc}_{k}", tag="h2o")
                         for k in range(4)]
                for of in range(10):
                    ps = mmp.tile([P, 512], F32)
                    for k in range(10):
                        nc.tensor.matmul(
                            ps[:], W2t[:, k, of * P:(of + 1) * P],
                            h1Tt[:, k, :], start=(k == 0), stop=(k == 9))
                    nc.scalar.activation(
                        h2Tt[:, of, :], ps[:],
                        mybir.ActivationFunctionType.Identity,
                        bias=b2t[:, of:of + 1])
                    # interleave: this of-chunk's exit transposes
                    for k in range(4):
                        pt = ptp.tile([P, P], F32R)
                        nc.tensor.transpose(
                            pt[:], h2Tt[:, of, k * P:(k + 1) * P], identt[:])
                        nc.vector.tensor_copy(
                            h2ot[k][:, of * P:(of + 1) * P], pt[:])
                    # interleave: next superchunk's entry transposes
                    if xgT_next is not None and of < 8:
                        for k in range(2):
                            entry_T(xg_next, xgT_next, of // 2, (of % 2) * 2 + k)
                for k in range(4):
                    r0 = sc * 512 + k * P
                    nc.sync.dma_start(h2_d.ap()[r0:r0 + P, :], h2ot[k][:])
                xg_cur, ea_cur, xgT_cur = xg_next, ea_next, xgT_next

        # ---------------- Phases S+N: scatter-mean + node MLP ----------------
        with ExitStack() as sctx:
            wpool2 = sctx.enter_context(tc.tile_pool(name="wN", bufs=1))
            W3t = wpool2.tile([P, 14, FN + FE], F32R)
            nc.sync.dma_start(
                W3t[:], W3_d.ap().rearrange("(ko ki) m -> ki ko m", ki=P))
            W4t = wpool2.tile([P, 5, FN], F32R)
            nc.sync.dma_start(
                W4t[:], W4_d.ap().rearrange("(ko ki) m -> ki ko m", ki=P))

            h2gp = sctx.enter_context(tc.tile_pool(name="h2g", bufs=7))
            Sp = sctx.enter_context(tc.tile_pool(name="Smat", bufs=3))
            aggp = sctx.enter_context(tc.tile_pool(name="agg", bufs=2))
            aggTp = sctx.enter_context(tc.tile_pool(name="aggT", bufs=2))
            xsp = sctx.enter_context(tc.tile_pool(name="xs", bufs=2))
            h3p = sctx.enter_context(tc.tile_pool(name="h3T", bufs=1))
            oTp = sctx.enter_context(tc.tile_pool(name="oT", bufs=2))
            ogp = sctx.enter_context(tc.tile_pool(name="og", bufs=4))
            smp = sctx.enter_context(
                tc.tile_pool(name="smp", bufs=1, space="PSUM"))
            mmp2 = sctx.enter_context(
                tc.tile_pool(name="mmN", bufs=2, space="PSUM"))
            ptp = sctx.enter_context(
                tc.tile_pool(name="ptp2", bufs=2, space="PSUM"))

            nj = (HID + 511) // 512  # psum 512-slices of the scatter output

            # Rolling gather lookahead: block b's h2-row gathers (slow,
            # gpsimd SW-DGE) are issued one block ahead of its scatter
            # matmuls so the PE never waits on them.  Pad slots carry an
            # out-of-bounds id and are silently skipped by the DMA
            # (bounds_check), so padding costs no gather bandwidth.
            pend_gs = {}

            def gather_S(b):
                lst = []
                for k in range(KB):
                    c = b * KB + k
                    h2g = h2gp.tile([P, HID], F32R, name=f"h2g_{b}_{k}",
                                    tag="h2g")
                    nc.gpsimd.indirect_dma_start(
                        out=h2g[:], out_offset=None, in_=h2_d.ap()[:],
                        in_offset=bass.IndirectOffsetOnAxis(
                            ap=gidt[:, c:c + 1], axis=0),
                        bounds_check=EP - 1, oob_is_err=False)
                    St = Sp.tile([P, P], F32R, name=f"S_{b}_{k}", tag="S")
                    nc.vector.tensor_tensor(
                        St[:], colbt[:, c:c + 1].to_broadcast([P, P]),
                        iotat[:], op=mybir.AluOpType.is_equal)
                    lst.append((h2g, St))
                pend_gs[b] = lst

            gather_S(0)
            gather_S(1)

            def load_xst(s):
                xst = xsp.tile([P, 4, 512], F32R, name=f"xst_{s}", tag="xst")
                nc.sync.dma_start(
                    xst[:],
                    xsT_d.ap().rearrange("(fo fi) n -> fi fo n", fi=P)
                    [:, :, s * 512:(s + 1) * 512])
                return xst

            xst_cur = load_xst(0)
            for s in range(NSB):
                aggTt = aggTp.tile([P, 10, 512], F32R)
                # pending aggT transposes: (agg_tile, bb) emitted lazily so
                # they interleave with the next block's scatter matmuls
                pend = []

                def emit_aggT(n):
                    for _ in range(n):
                        if not pend:
                            return
                        agg, bb2, f = pend.pop(0)
                        pt = ptp.tile([P, P], F32R)
                        nc.tensor.transpose(
                            pt[:], agg[:, f * P:(f + 1) * P], identt[:])
                        nc.vector.tensor_copy(
                            aggTt[:, f, bb2 * P:(bb2 + 1) * P], pt[:])

                for bb in range(4):
                    b = s * 4 + bb
                    if b + 2 < NB:
                        gather_S(b + 2)
                    pss = smp.tile([P, HID], F32)
                    for k, (h2g, St) in enumerate(pend_gs.pop(b)):
                        for j in range(nj):
                            lo, hi = j * 512, min((j + 1) * 512, HID)
                            nc.tensor.matmul(
                                pss[:, lo:hi], St[:], h2g[:, lo:hi],
                                start=(k == 0), stop=(k == KB - 1))
                        q = 10 // KB
                        emit_aggT(10 - (KB - 1) * q if k == KB - 1 else q)
                    agg = aggp.tile([P, HID], F32R)
                    nc.scalar.activation(
                        agg[:], pss[:], mybir.ActivationFunctionType.Copy,
                        bias=0.0, scale=invct[:, b:b + 1])
                    pend.extend((agg, bb, f) for f in range(10))

                xst = xst_cur
                xst_cur = load_xst(s + 1) if s + 1 < NSB else None
                h3Tt = h3p.tile([P, 5, 512], F32R)
                for of in range(5):
                    ps = mmp2.tile([P, 512], F32)
                    for k in range(4):
                        nc.tensor.matmul(
                            ps[:], W3t[:, k, of * P:(of + 1) * P],
                            xst[:, k, :], start=(k == 0), stop=False)
                        emit_aggT(3)  # last block's transposes, staggered
                    for f in range(10):
                        nc.tensor.matmul(
                            ps[:], W3t[:, 4 + f, of * P:(of + 1) * P],
                            aggTt[:, f, :], start=False, stop=(f == 9))
                    nc.scalar.activation(h3Tt[:, of, :], ps[:], RELU,
                                         bias=b3t[:, of:of + 1])
                emit_aggT(100)  # drain any stragglers (non-standard KB)
                oTt = oTp.tile([P, 4, 512], F32R)
                ogs = [ogp.tile([P, FN], F32, name=f"og_{s}_{g}", tag="og")
                       for g in range(4)]
                for of in range(4):
                    ps = mmp2.tile([P, 512], F32)
                    for k in range(5):
                        nc.tensor.matmul(
                            ps[:], W4t[:, k, of * P:(of + 1) * P],
                            h3Tt[:, k, :], start=(k == 0), stop=(k == 4))
                    nc.scalar.activation(
                        oTt[:, of, :], ps[:],
                        mybir.ActivationFunctionType.Identity,
                        bias=b4t[:, of:of + 1])
                    for g in range(4):
                        pt = ptp.tile([P, P], F32R)
                        nc.tensor.transpose(
                            pt[:], oTt[:, of, g * P:(g + 1) * P], identt[:])
                        nc.vector.tensor_copy(
                            ogs[g][:, of * P:(of + 1) * P],
                            pt[:].bitcast(F32))
                for g in range(4):
                    r0 = s * 512 + g * P
                    nc.sync.dma_start(out_d.ap()[r0:r0 + P, :], ogs[g][:])

    nc.compile()
    return nc


def _prepare(x, row, col, ea):
    """Host-side sharding: sort edges by destination, split nodes into 8
    block-aligned edge-balanced shards, build per-core arrays."""
    N = x.shape[0]
    E = ea.shape[0]
    order = np.argsort(col, kind="stable")
    scol = col[order]
    srow = row[order]
    NBLK = (N + P - 1) // P
    NTOT = NBLK * P

    bounds = [0]
    for p in range(1, NCORES):
        if E > 0:
            t = int(scol[min((p * E) // NCORES, E - 1)])
        else:
            t = (p * NTOT) // NCORES
        b = int(round(t / P)) * P
        b = max(b, bounds[-1] + P)
        b = min(b, NTOT - P * (NCORES - p))
        bounds.append(b)
    bounds.append(NTOT)
    for p in range(1, NCORES + 1):
        assert bounds[p] > bounds[p - 1], f"degenerate shard bounds {bounds}"

    e_split = np.searchsorted(scol, bounds)
    Ec = np.diff(e_split)
    EC = max(4, math.ceil(int(Ec.max()) / P))
    EC = ((EC + 3) // 4) * 4
    EP = EC * P
    nblk = [(bounds[p + 1] - bounds[p]) // P for p in range(NCORES)]
    NB = max(4, ((max(nblk) + 3) // 4) * 4)
    NBP = NB * P
    blkdeg = np.bincount(scol // P, minlength=NBLK)
    KB = max(1, math.ceil(int(blkdeg.max()) / P))

    xpadT = np.zeros((FN, NTOT + NBP), np.float32)
    xpadT[:, :N] = x.T

    cores = []
    for p in range(NCORES):
        s, e = int(e_split[p]), int(e_split[p + 1])
        n0 = bounds[p]
        ne = e - s
        tmp = np.zeros(EP, np.int32)
        tmp[:ne] = srow[s:e]
        rows_t = np.ascontiguousarray(tmp.reshape(EC, P).T)
        eaT = np.zeros((FE, EP), np.float32)
        eaT[:, :ne] = ea[order[s:e]].T
        lcol = (scol[s:e] - n0).astype(np.int64)
        bstart = np.searchsorted(lcol, np.arange(NB + 1) * P)
        gid = np.full((NB, KB, P), 1 << 30, np.int32)
        gid.reshape(NB * KB, P)[:7] = 0
        colb = np.full((NB, KB, P), -1.0, np.float32)
        for b in range(NB):
            sb, eb = int(bstart[b]), int(bstart[b + 1])
            cnt = eb - sb
            assert cnt <= KB * P
            gid[b].reshape(-1)[:cnt] = np.arange(sb, eb, dtype=np.int32)
            colb[b].reshape(-1)[:cnt] = (lcol[sb:eb] - b * P)
        gid_t = np.ascontiguousarray(gid.reshape(NB * KB, P).T)
        colb_t = np.ascontiguousarray(colb.reshape(NB * KB, P).T)
        deg = np.bincount(lcol, minlength=NBP)[:NBP]
        invc_t = np.ascontiguousarray(
            (1.0 / np.maximum(deg, 1.0)).astype(np.float32).reshape(NB, P).T)
        xsT = np.ascontiguousarray(xpadT[:, n0:n0 + NBP])
        cores.append(dict(rows=rows_t, eaT=eaT, gid=gid_t, colb=colb_t,
                          invc=invc_t, xsT=xsT))
    return cores, bounds, EC, NB, KB


def _run(inputs, trace=False):
    x = np.ascontiguousarray(np.asarray(inputs["x"], dtype=np.float32))
    ei = np.asarray(inputs["edge_index"])
    ea = np.ascontiguousarray(np.asarray(inputs["edge_attr"], dtype=np.float32))
    row = ei[0].astype(np.int64)
    col = ei[1].astype(np.int64)
    W1 = np.ascontiguousarray(np.asarray(inputs["W1"], np.float32))
    W2 = np.ascontiguousarray(np.asarray(inputs["W2"], np.float32))
    W3 = np.ascontiguousarray(np.asarray(inputs["W3"], np.float32))
    W4 = np.ascontiguousarray(np.asarray(inputs["W4"], np.float32))
    b1 = np.asarray(inputs["b1"], np.float32)
    b2 = np.asarray(inputs["b2"], np.float32)
    b3 = np.asarray(inputs["b3"], np.float32)
    b4 = np.asarray(inputs["b4"], np.float32)
    N = x.shape[0]

    cores, bounds, EC, NB, KB = _prepare(x, row, col, ea)

    key = (EC, NB, KB, N)
    if key not in _prog_cache:
        _prog_cache[key] = _build(EC, NB, KB, N)
    nc = _prog_cache[key]

    b1t = np.ascontiguousarray(b1.reshape(HID // P, P).T)
    b2t = np.ascontiguousarray(b2.reshape(HID // P, P).T)
    b3t = np.ascontiguousarray(b3.reshape((FN + FE) // P, P).T)
    b4t = np.ascontiguousarray(b4.reshape(FN // P, P).T)
    iota = np.ascontiguousarray(
        np.broadcast_to(np.arange(P, dtype=np.float32), (P, P)))
    ident = np.eye(P, dtype=np.float32)

    in_maps = []
    for p in range(NCORES):
        c = cores[p]
        in_maps.append({
            "x": x, "rows": c["rows"], "eaT": c["eaT"],
            "W1": W1, "W2": W2, "W3": W3, "W4": W4,
            "b1": b1t, "b2": b2t, "b3": b3t, "b4": b4t,
            "gid": c["gid"], "colb": c["colb"], "invc": c["invc"],
            "xsT": c["xsT"], "iota": iota, "ident": ident,
        })

    res = run_bass_kernel_spmd(nc, in_maps, list(range(NCORES)), trace=trace)

    out = np.empty((N, FN), np.float32)
    for p in range(NCORES):
        n0, n1 = bounds[p], min(bounds[p + 1], N)
        if n1 > n0:
            out[n0:n1] = res.results[p]["out"][:n1 - n0]
    return out, res


def kernel(**inputs) -> np.ndarray:
    out, _ = _run(inputs, trace=False)
    return out



# revision 9
# speedup vs baseline: 1.3283x; 1.3283x over previous
"""Trainium2 Bass kernel for nn_NodeModel (GNN message passing), bf16.

Reference computation:
    h   = relu(concat(x[row], edge_attr) @ W1 + b1) @ W2 + b2     # edge MLP
    agg = scatter_mean(h, col, N)                                  # per-dest mean
    out = relu(concat(x, agg) @ W3 + b3) @ W4 + b4                 # node MLP

Distribution strategy (8 cores, no collectives):
  - Sort edges by destination node; split destination nodes into 8
    block-aligned, edge-balanced shards.  Each core owns one node shard and
    ALL edges targeting it, so per-node sums are complete locally.
  - The gathered-and-transposed edge input concat(x[row], edge_attr)^T is
    prepared host-side per core, so the device runs no transposes at all:
      * h1 is computed in [hid1, edge] layout (weights stationary),
      * h2 directly in [edge, hid2] row layout (h1T slices stationary,
        W2 moving) and staged pre-scaled by 1/deg(dest) to DRAM,
      * scatter-mean gathers h2 rows per 128-node block and matmuls
        h2_slice^T @ onehot(S) giving agg^T [hid, node] directly,
      * node MLP consumes agg^T and x^T and emits the output in [node,
        feat] row layout (h3T slices stationary, W4 moving).
  - All matmuls run in bfloat16 (1 cycle/row on the PE, like fp32r, but
    half the SBUF/DMA traffic and no transpose/fp32r throttle penalties);
    accumulation is fp32 in PSUM.  fp8 was measured numerically out of
    reach for the 2e-2 gate (any single stage in fp8e4m3 costs ~2e-2).
"""

import math
import sys
from contextlib import ExitStack

sys.path.insert(0, "/opt/trn_rl_repo")

import numpy as np
import ml_dtypes

import concourse.bass as bass
import concourse.tile as tile
from concourse import bacc, mybir
from concourse.bass_utils import run_bass_kernel_spmd

NCORES = 8
P = 128
FN = 512    # node feature dim
FE = 128    # edge feature dim
HID = 1280  # edge-MLP hidden/output dim
IN1 = FN + FE          # 640
IN2 = FN + HID         # 1792
BF16 = mybir.dt.bfloat16
F32 = mybir.dt.float32
I32 = mybir.dt.int32
RELU = mybir.ActivationFunctionType.Relu
COPY = mybir.ActivationFunctionType.Copy
NPBF = ml_dtypes.bfloat16

_prog_cache = {}


def _build(EC, NB, KB, use_b2, use_b4):
    """Build the SPMD program for one core.

    EC: edge chunks (128 edges each) per core, multiple of 4.
    NB: node blocks (128 nodes each) per core, multiple of 4.
    KB: max edge chunks per node block (scatter schedule width).
    """
    EP = EC * P
    NBP = NB * P
    SC = EC // 4   # superchunks of 512 edges
    NSB = NB // 4  # superblocks of 512 nodes

    nc = bacc.Bacc("TRN2", target_bir_lowering=False, debug=False,
                   num_devices=NCORES)

    inT_d = nc.dram_tensor("inT", [P, 5, EP], BF16, kind="ExternalInput")
    W1_d = nc.dram_tensor("W1", [IN1, HID], BF16, kind="ExternalInput")
    W2_d = nc.dram_tensor("W2", [HID, HID], BF16, kind="ExternalInput")
    W3_d = nc.dram_tensor("W3", [IN2, IN1], BF16, kind="ExternalInput")
    W4_d = nc.dram_tensor("W4", [IN1, FN], BF16, kind="ExternalInput")
    b1_d = nc.dram_tensor("b1", [P, HID // P], F32, kind="ExternalInput")
    b3_d = nc.dram_tensor("b3", [P, IN1 // P], F32, kind="ExternalInput")
    gid_d = nc.dram_tensor("gid", [P, NB * KB], I32, kind="ExternalInput")
    colb_d = nc.dram_tensor("colb", [P, NB * KB], F32, kind="ExternalInput")
    invce_d = nc.dram_tensor("invce", [P, EC], F32, kind="ExternalInput")
    xsT_d = nc.dram_tensor("xsT", [P, 4, NBP], BF16, kind="ExternalInput")
    iota_d = nc.dram_tensor("iota", [P, P], F32, kind="ExternalInput")
    if use_b2:
        b2r_d = nc.dram_tensor("b2r", [1, HID], BF16, kind="ExternalInput")
        m2r_d = nc.dram_tensor("m2r", [1, NBP], BF16, kind="ExternalInput")
    if use_b4:
        b4r_d = nc.dram_tensor("b4r", [1, FN], BF16, kind="ExternalInput")
        onesr_d = nc.dram_tensor("onesr", [1, P], BF16, kind="ExternalInput")
    out_d = nc.dram_tensor("out", [NBP, FN], F32, kind="ExternalOutput")
    h2_d = nc.dram_tensor("h2buf", [EP, HID], BF16)  # internal staging

    with tile.TileContext(nc) as tc, ExitStack() as ctx:
        cpool = ctx.enter_context(tc.tile_pool(name="const", bufs=1))

        iotat = cpool.tile([P, P], F32)
        nc.sync.dma_start(iotat[:], iota_d.ap()[:])
        b1t = cpool.tile([P, HID // P], F32)
        nc.sync.dma_start(b1t[:], b1_d.ap()[:])
        b3t = cpool.tile([P, IN1 // P], F32)
        nc.sync.dma_start(b3t[:], b3_d.ap()[:])
        gidt = cpool.tile([P, NB * KB], I32)
        nc.sync.dma_start(gidt[:], gid_d.ap()[:])
        colbt = cpool.tile([P, NB * KB], F32)
        nc.sync.dma_start(colbt[:], colb_d.ap()[:])
        invct = cpool.tile([P, EC], F32)
        nc.sync.dma_start(invct[:], invce_d.ap()[:])
        W1t = cpool.tile([P, 5, HID], BF16)
        W1r = W1_d.ap().rearrange("(ko ki) m -> ki ko m", ki=P)
        for k in range(5):
            nc.sync.dma_start(W1t[:, k, :], W1r[:, k, :])
        W2t = cpool.tile([P, 10, HID], BF16)
        W2r = W2_d.ap().rearrange("(ko ki) m -> ki ko m", ki=P)
        for k in range(10):
            nc.sync.dma_start(W2t[:, k, :], W2r[:, k, :])
        W3t = cpool.tile([P, 14, IN1], BF16)
        nc.sync.dma_start(
            W3t[:], W3_d.ap().rearrange("(ko ki) m -> ki ko m", ki=P))
        W4t = cpool.tile([P, 5, FN], BF16)
        nc.sync.dma_start(
            W4t[:], W4_d.ap().rearrange("(ko ki) m -> ki ko m", ki=P))
        if use_b2:
            b2rt = cpool.tile([1, HID], BF16)
            nc.sync.dma_start(b2rt[:], b2r_d.ap()[:])
            m2rt = cpool.tile([1, NBP], BF16)
            nc.sync.dma_start(m2rt[:], m2r_d.ap()[:])
        if use_b4:
            b4rt = cpool.tile([1, FN], BF16)
            nc.sync.dma_start(b4rt[:], b4r_d.ap()[:])
            onesrt = cpool.tile([1, P], BF16)
            nc.sync.dma_start(onesrt[:], onesr_d.ap()[:])

        # ---------------- Phase E: edge MLP ----------------
        with ExitStack() as ectx:
            inp = ectx.enter_context(tc.tile_pool(name="inT", bufs=2))
            h1p = ectx.enter_context(tc.tile_pool(name="h1T", bufs=2))
            h2p = ectx.enter_context(tc.tile_pool(name="h2r", bufs=4))
            mmp = ectx.enter_context(
                tc.tile_pool(name="mmE", bufs=2, space="PSUM"))

            def load_in(sc):
                t = inp.tile([P, 5, 512], BF16, name=f"in_{sc}", tag="inT")
                nc.sync.dma_start(t[:], inT_d.ap()[:, :, sc * 512:(sc + 1) * 512])
                return t

            in_cur = load_in(0)
            for sc in range(SC):
                in_next = load_in(sc + 1) if sc + 1 < SC else None
                # h1T [hid1, e]: stationary W1 slices, moving inT
                h1Tt = h1p.tile([P, 10, 512], BF16)
                for of in range(10):
                    ps = mmp.tile([P, 512], F32)
                    for k in range(5):
                        nc.tensor.matmul(
                            ps[:], W1t[:, k, of * P:(of + 1) * P],
                            in_cur[:, k, :], start=(k == 0), stop=(k == 4))
                    nc.scalar.activation(h1Tt[:, of, :], ps[:], RELU,
                                         bias=b1t[:, of:of + 1])
                # h2 [e, hid2] rows: stationary h1T chunk slices, moving W2
                for c in range(4):
                    h2t = h2p.tile([P, HID], BF16, name=f"h2_{sc}_{c}",
                                   tag="h2r")
                    for hs in range(3):
                        lo = hs * 512
                        hi = min(lo + 512, HID)
                        ps = mmp.tile([P, hi - lo], F32)
                        for k in range(10):
                            nc.tensor.matmul(
                                ps[:], h1Tt[:, k, c * P:(c + 1) * P],
                                W2t[:, k, lo:hi], start=(k == 0), stop=(k == 9))
                        # scale by 1/deg(dest) per edge (partition) + bf16 cast
                        nc.vector.tensor_scalar(
                            h2t[:, lo:hi], ps[:],
                            invct[:, sc * 4 + c:sc * 4 + c + 1], None,
                            op0=mybir.AluOpType.mult)
                    r0 = sc * 512 + c * P
                    nc.sync.dma_start(h2_d.ap()[r0:r0 + P, :], h2t[:])
                in_cur = in_next

        # ---------------- Phases S+N: scatter-mean + node MLP ----------------
        with ExitStack() as sctx:
            h2gp = sctx.enter_context(tc.tile_pool(name="h2g", bufs=3 * KB))
            Sp = sctx.enter_context(tc.tile_pool(name="Smat", bufs=3 * KB))
            aggp = sctx.enter_context(tc.tile_pool(name="aggT", bufs=2))
            xsp = sctx.enter_context(tc.tile_pool(name="xs", bufs=2))
            h3p = sctx.enter_context(tc.tile_pool(name="h3T", bufs=2))
            ogp = sctx.enter_context(tc.tile_pool(name="og", bufs=4))
            spp = sctx.enter_context(
                tc.tile_pool(name="spp", bufs=2, space="PSUM"))
            mmp2 = sctx.enter_context(
                tc.tile_pool(name="mmN", bufs=2, space="PSUM"))

            pend_gs = {}

            def gather_S(b):
                lst = []
                for k in range(KB):
                    c = b * KB + k
                    h2g = h2gp.tile([P, HID], BF16, name=f"h2g_{b}_{k}",
                                    tag="h2g")
                    nc.gpsimd.indirect_dma_start(
                        out=h2g[:], out_offset=None, in_=h2_d.ap()[:],
                        in_offset=bass.IndirectOffsetOnAxis(
                            ap=gidt[:, c:c + 1], axis=0),
                        bounds_check=EP - 1, oob_is_err=False)
                    St = Sp.tile([P, P], BF16, name=f"S_{b}_{k}", tag="S")
                    nc.vector.tensor_tensor(
                        St[:], colbt[:, c:c + 1].to_broadcast([P, P]),
                        iotat[:], op=mybir.AluOpType.is_equal)
                    lst.append((h2g, St))
                pend_gs[b] = lst

            gather_S(0)
            gather_S(1)
            z512 = cpool.tile([1, 512], BF16)
            nc.vector.memset(z512[:], 0.0)

            def load_xst(s):
                t = xsp.tile([P, 4, 512], BF16, name=f"xst_{s}", tag="xst")
                nc.sync.dma_start(
                    t[:], xsT_d.ap()[:, :, s * 512:(s + 1) * 512])
                return t

            xst_cur = load_xst(0)
            for s in range(NSB):
                aggTt = aggp.tile([P, 10, 512], BF16)
                for bb in range(4):
                    b = s * 4 + bb
                    if b + 2 < NB:
                        gather_S(b + 2)
                    gs = pend_gs.pop(b)
                    # The 10 per-block accumulation groups are 512B each —
                    # sub-bank.  start_tensor_calc zeroes whole 2KB PSUM
                    # banks, so interleaved per-group starts corrupt bank
                    # neighbours.  Instead zero the full 2.5-bank tile with
                    # three K=1 bank-covering matmuls, then accumulate all
                    # real matmuls with start=False.
                    sp = spp.tile([P, 10 * P], F32)
                    for z0 in range(0, 10 * P, 512):
                        zw = min(512, 10 * P - z0)
                        nc.tensor.matmul(
                            sp[:, z0:z0 + zw], z512[0:1, 0:P],
                            z512[0:1, 0:zw], start=True, stop=False)
                    if use_b2:
                        # b2 masked by (deg>0): out[m,n] = b2[m]*mask[n]
                        for hs in range(10):
                            nc.tensor.matmul(
                                sp[:, hs * P:(hs + 1) * P],
                                b2rt[0:1, hs * P:(hs + 1) * P],
                                m2rt[0:1, b * P:(b + 1) * P],
                                start=False, stop=False)
                    for k, (h2g, St) in enumerate(gs):
                        for hs in range(10):
                            nc.tensor.matmul(
                                sp[:, hs * P:(hs + 1) * P],
                                h2g[:, hs * P:(hs + 1) * P],
                                St[:], start=False, stop=(k == KB - 1))
                    for hs in range(10):
                        nc.scalar.activation(
                            aggTt[:, hs, bb * P:(bb + 1) * P],
                            sp[:, hs * P:(hs + 1) * P], COPY)

                xst = xst_cur
                xst_cur = load_xst(s + 1) if s + 1 < NSB else None
                # h3T [of, n]: stationary W3 slices, moving xsT/aggT
                h3Tt = h3p.tile([P, 5, 512], BF16)
                for of in range(5):
                    ps = mmp2.tile([P, 512], F32)
                    for k in range(4):
                        nc.tensor.matmul(
                            ps[:], W3t[:, k, of * P:(of + 1) * P],
                            xst[:, k, :], start=(k == 0), stop=False)
                    for k in range(10):
                        nc.tensor.matmul(
                            ps[:], W3t[:, 4 + k, of * P:(of + 1) * P],
                            aggTt[:, k, :], start=False, stop=(k == 9))
                    nc.scalar.activation(h3Tt[:, of, :], ps[:], RELU,
                                         bias=b3t[:, of:of + 1])
                # out [n, feat] rows: stationary h3T slices, moving W4
                for c in range(4):
                    ps = mmp2.tile([P, FN], F32)
                    for k in range(5):
                        nc.tensor.matmul(
                            ps[:], h3Tt[:, k, c * P:(c + 1) * P],
                            W4t[:, k, :], start=(k == 0),
                            stop=(k == 4 and not use_b4))
                    if use_b4:
                        nc.tensor.matmul(
                            ps[:], onesrt[0:1, :], b4rt[0:1, :],
                            start=False, stop=True)
                    og = ogp.tile([P, FN], F32, name=f"og_{s}_{c}", tag="og")
                    nc.scalar.activation(og[:], ps[:], COPY)
                    r0 = s * 512 + c * P
                    nc.sync.dma_start(out_d.ap()[r0:r0 + P, :], og[:])

    nc.compile()
    return nc


def _prepare(x8, row, col, ea8):
    """Host-side sharding: sort edges by destination, split nodes into 8
    block-aligned edge-balanced shards, build per-core arrays (bf16)."""
    N = x8.shape[0]
    E = ea8.shape[0]
    order = np.argsort(col, kind="stable")
    scol = col[order]
    srow = row[order]
    NBLK = (N + P - 1) // P
    NTOT = NBLK * P

    bounds = [0]
    for p in range(1, NCORES):
        if E > 0:
            t = int(scol[min((p * E) // NCORES, E - 1)])
        else:
            t = (p * NTOT) // NCORES
        b = int(round(t / P)) * P
        b = max(b, bounds[-1] + P)
        b = min(b, NTOT - P * (NCORES - p))
        bounds.append(b)
    bounds.append(NTOT)
    for p in range(1, NCORES + 1):
        assert bounds[p] > bounds[p - 1], f"degenerate shard bounds {bounds}"

    e_split = np.searchsorted(scol, bounds)
    Ec = np.diff(e_split)
    EC = max(4, math.ceil(int(Ec.max()) / P))
    EC = ((EC + 3) // 4) * 4
    EP = EC * P
    nblk = [(bounds[p + 1] - bounds[p]) // P for p in range(NCORES)]
    NB = max(4, ((max(nblk) + 3) // 4) * 4)
    NBP = NB * P
    blkdeg = np.bincount(scol // P, minlength=NBLK)
    KB = max(1, math.ceil(int(blkdeg.max()) / P))

    deg = np.bincount(scol, minlength=NTOT + NBP).astype(np.float32)
    inve_all = 1.0 / np.maximum(deg[scol], 1.0)  # per sorted edge

    xpadT = np.zeros((FN, NTOT + NBP), NPBF)
    xpadT[:, :N] = x8.T

    cores = []
    for p in range(NCORES):
        s, e = int(e_split[p]), int(e_split[p + 1])
        n0 = bounds[p]
        ne = e - s
        # gathered+transposed edge-MLP input [ki, ko, e]
        feat = np.zeros((EP, IN1), NPBF)
        feat[:ne, :FN] = x8[srow[s:e]]
        feat[:ne, FN:] = ea8[order[s:e]]
        inT = np.ascontiguousarray(
            feat.T.reshape(5, P, EP).transpose(1, 0, 2))
        # per-edge 1/deg(dest) in [ki, chunk] layout
        ive = np.zeros(EP, np.float32)
        ive[:ne] = inve_all[s:e]
        invce = np.ascontiguousarray(ive.reshape(EC, P).T)
        # scatter schedule
        lcol = (scol[s:e] - n0).astype(np.int64)
        bstart = np.searchsorted(lcol, np.arange(NB + 1) * P)
        gid = np.full((NB, KB, P), 1 << 30, np.int32)
        gid.reshape(NB * KB, P)[:3 * KB] = 0  # first tiles: finite data
        colb = np.full((NB, KB, P), -1.0, np.float32)
        for b in range(NB):
            sb, eb = int(bstart[b]), int(bstart[b + 1])
            cnt = eb - sb
            assert cnt <= KB * P
            gid[b].reshape(-1)[:cnt] = np.arange(sb, eb, dtype=np.int32)
            colb[b].reshape(-1)[:cnt] = (lcol[sb:eb] - b * P)
        gid_t = np.ascontiguousarray(gid.reshape(NB * KB, P).T)
        colb_t = np.ascontiguousarray(colb.reshape(NB * KB, P).T)
        xsT = np.ascontiguousarray(
            xpadT[:, n0:n0 + NBP].reshape(4, P, NBP).transpose(1, 0, 2))
        ndeg = deg[n0:n0 + NBP]
        cores.append(dict(inT=inT, invce=invce, gid=gid_t, colb=colb_t,
                          xsT=xsT, ndeg=ndeg))
    return cores, bounds, EC, NB, KB


def _run(inputs, trace=False):
    x = np.asarray(inputs["x"], dtype=np.float32)
    ei = np.asarray(inputs["edge_index"])
    ea = np.asarray(inputs["edge_attr"], dtype=np.float32)
    row = ei[0].astype(np.int64)
    col = ei[1].astype(np.int64)
    x8 = x.astype(NPBF)
    ea8 = ea.astype(NPBF)
    W1 = np.ascontiguousarray(np.asarray(inputs["W1"], np.float32)).astype(NPBF)
    W2 = np.ascontiguousarray(np.asarray(inputs["W2"], np.float32)).astype(NPBF)
    W3 = np.ascontiguousarray(np.asarray(inputs["W3"], np.float32)).astype(NPBF)
    W4 = np.ascontiguousarray(np.asarray(inputs["W4"], np.float32)).astype(NPBF)
    b1 = np.asarray(inputs["b1"], np.float32)
    b2 = np.asarray(inputs["b2"], np.float32)
    b3 = np.asarray(inputs["b3"], np.float32)
    b4 = np.asarray(inputs["b4"], np.float32)
    N = x.shape[0]

    cores, bounds, EC, NB, KB = _prepare(x8, row, col, ea8)
    use_b2 = bool(np.any(b2))
    use_b4 = bool(np.any(b4))

    key = (EC, NB, KB, use_b2, use_b4)
    if key not in _prog_cache:
        _prog_cache[key] = _build(EC, NB, KB, use_b2, use_b4)
    nc = _prog_cache[key]

    b1t = np.ascontiguousarray(b1.reshape(HID // P, P).T)
    b3t = np.ascontiguousarray(b3.reshape(IN1 // P, P).T)
    iota = np.ascontiguousarray(
        np.broadcast_to(np.arange(P, dtype=np.float32), (P, P)))

    in_maps = []
    for p in range(NCORES):
        c = cores[p]
        m = {
            "inT": c["inT"], "W1": W1, "W2": W2, "W3": W3, "W4": W4,
            "b1": b1t, "b3": b3t, "gid": c["gid"], "colb": c["colb"],
            "invce": c["invce"], "xsT": c["xsT"], "iota": iota,
        }
        if use_b2:
            m["b2r"] = np.ascontiguousarray(b2.reshape(1, HID)).astype(NPBF)
            m["m2r"] = (c["ndeg"] > 0).reshape(1, -1).astype(NPBF)
        if use_b4:
            m["b4r"] = np.ascontiguousarray(b4.reshape(1, FN)).astype(NPBF)
            m["onesr"] = np.ones((1, P), NPBF)
        in_maps.append(m)

    res = run_bass_kernel_spmd(nc, in_maps, list(range(NCORES)), trace=trace)

    out = np.empty((N, FN), np.float32)
    for p in range(NCORES):
        n0, n1 = bounds[p], min(bounds[p + 1], N)
        if n1 > n0:
            out[n0:n1] = res.results[p]["out"][:n1 - n0]
    return out, res


def kernel(**inputs) -> np.ndarray:
    out, _ = _run(inputs, trace=False)
    return out


# revision 10
# speedup vs baseline: 1.6551x; 1.2460x over previous
"""Trainium2 Bass kernel for nn_NodeModel (GNN message passing), bf16.

Reference computation:
    h   = relu(concat(x[row], edge_attr) @ W1 + b1) @ W2 + b2     # edge MLP
    agg = scatter_mean(h, col, N)                                  # per-dest mean
    out = relu(concat(x, agg) @ W3 + b3) @ W4 + b4                 # node MLP

Key algebraic restructure: scatter_mean is linear, so W2 commutes with it:
    agg = scatter_mean(relu(h1), col) @ W2 + b2*[deg>0]
which applies the 1280x1280 W2 matmul per NODE (~6.3k rows/core) instead
of per EDGE (~16k rows/core) — a ~2.5x FLOP cut on the largest matmul.

Distribution strategy (8 cores, no collectives):
  - Sort edges by destination node; split destination nodes into 8
    block-aligned, edge-balanced shards.  Each core owns one node shard and
    ALL edges targeting it, so per-node means are complete locally.
  - The gathered-and-transposed edge input concat(x[row], edge_attr)^T is
    prepared host-side per core, so the device runs no transposes at all:
      * h1 rows [edge, 1280] are computed directly (inT chunk slices
        stationary, W1 moving), relu'd and pre-scaled by 1/deg(dest) at
        PSUM drain, staged to DRAM,
      * scatter-mean gathers the rows of each 128-node destination block
        and matmuls h1_slice^T @ onehot(S), accumulating agg^T directly,
      * aggT = W2^T @ aggH1T per superblock (W2 slices stationary),
      * node MLP consumes agg^T and x^T and emits output rows directly.
  - All matmuls run in bfloat16 (1 cycle/row on the PE, like fp32r, but
    half the SBUF/DMA traffic, no fp32r small-free-dim penalty, and no
    transpose/HAM-throttle); accumulation is fp32 in PSUM.  fp8 was
    measured numerically out of reach for the 2e-2 gate.
  - PSUM accumulation groups are kept bank-safe: the 10 per-block scatter
    groups (512B each, sub-bank) are zero-initialized by three K=1
    bank-covering matmuls, and every real matmul accumulates with
    start=False (start_tensor_calc zeroes whole 2KB banks and would
    corrupt bank neighbours).
"""

import math
import sys
from contextlib import ExitStack

sys.path.insert(0, "/opt/trn_rl_repo")

import numpy as np
import ml_dtypes

import concourse.bass as bass
import concourse.tile as tile
from concourse import bacc, mybir
from concourse.bass_utils import run_bass_kernel_spmd

NCORES = 8
P = 128
FN = 512    # node feature dim
FE = 128    # edge feature dim
HID = 1280  # edge-MLP hidden/output dim
IN1 = FN + FE          # 640
IN2 = FN + HID         # 1792
BF16 = mybir.dt.bfloat16
F32 = mybir.dt.float32
I32 = mybir.dt.int32
RELU = mybir.ActivationFunctionType.Relu
COPY = mybir.ActivationFunctionType.Copy
NPBF = ml_dtypes.bfloat16

_prog_cache = {}


def _build(EC, NB, KB, use_b1, use_b2, use_b4):
    """Build the SPMD program for one core.

    EC: edge chunks (128 edges each) per core, multiple of 4.
    NB: node blocks (128 nodes each) per core, multiple of 4.
    KB: max edge chunks per node block (scatter schedule width).
    """
    EP = EC * P
    NBP = NB * P
    SC = EC // 4   # superchunks of 512 edges
    NSB = NB // 4  # superblocks of 512 nodes

    nc = bacc.Bacc("TRN2", target_bir_lowering=False, debug=False,
                   num_devices=NCORES)

    inT_d = nc.dram_tensor("inT", [P, 5, EP], BF16, kind="ExternalInput")
    W1_d = nc.dram_tensor("W1", [IN1, HID], BF16, kind="ExternalInput")
    W2_d = nc.dram_tensor("W2", [HID, HID], BF16, kind="ExternalInput")
    W3_d = nc.dram_tensor("W3", [IN2, IN1], BF16, kind="ExternalInput")
    W4_d = nc.dram_tensor("W4", [IN1, FN], BF16, kind="ExternalInput")
    b3_d = nc.dram_tensor("b3", [P, IN1 // P], F32, kind="ExternalInput")
    gid_d = nc.dram_tensor("gid", [P, NB * KB], I32, kind="ExternalInput")
    colb_d = nc.dram_tensor("colb", [P, NB * KB], F32, kind="ExternalInput")
    invce_d = nc.dram_tensor("invce", [P, EC], F32, kind="ExternalInput")
    xsT_d = nc.dram_tensor("xsT", [P, 4, NBP], BF16, kind="ExternalInput")
    iota_d = nc.dram_tensor("iota", [P, P], F32, kind="ExternalInput")
    if use_b1:
        b1r_d = nc.dram_tensor("b1r", [P, HID], F32, kind="ExternalInput")
    if use_b2:
        b2r_d = nc.dram_tensor("b2r", [1, HID], BF16, kind="ExternalInput")
        m2r_d = nc.dram_tensor("m2r", [1, NBP], BF16, kind="ExternalInput")
    if use_b4:
        b4r_d = nc.dram_tensor("b4r", [1, FN], BF16, kind="ExternalInput")
        onesr_d = nc.dram_tensor("onesr", [1, P], BF16, kind="ExternalInput")
    out_d = nc.dram_tensor("out", [NBP, FN], F32, kind="ExternalOutput")
    h1_d = nc.dram_tensor("h1buf", [EP, HID], BF16)  # internal staging

    with tile.TileContext(nc) as tc, ExitStack() as ctx:
        cpool = ctx.enter_context(tc.tile_pool(name="const", bufs=1))

        iotat = cpool.tile([P, P], F32)
        nc.sync.dma_start(iotat[:], iota_d.ap()[:])
        b3t = cpool.tile([P, IN1 // P], F32)
        nc.sync.dma_start(b3t[:], b3_d.ap()[:])
        gidt = cpool.tile([P, NB * KB], I32)
        nc.sync.dma_start(gidt[:], gid_d.ap()[:])
        colbt = cpool.tile([P, NB * KB], F32)
        nc.sync.dma_start(colbt[:], colb_d.ap()[:])
        invct = cpool.tile([P, EC], F32)
        nc.sync.dma_start(invct[:], invce_d.ap()[:])
        W1t = cpool.tile([P, 5, HID], BF16)
        W1r = W1_d.ap().rearrange("(ko ki) m -> ki ko m", ki=P)
        for k in range(5):
            nc.sync.dma_start(W1t[:, k, :], W1r[:, k, :])
        W2t = cpool.tile([P, 10, HID], BF16)
        W2r = W2_d.ap().rearrange("(ko ki) m -> ki ko m", ki=P)
        for k in range(10):
            nc.sync.dma_start(W2t[:, k, :], W2r[:, k, :])
        W3t = cpool.tile([P, 14, IN1], BF16)
        nc.sync.dma_start(
            W3t[:], W3_d.ap().rearrange("(ko ki) m -> ki ko m", ki=P))
        W4t = cpool.tile([P, 5, FN], BF16)
        nc.sync.dma_start(
            W4t[:], W4_d.ap().rearrange("(ko ki) m -> ki ko m", ki=P))
        if use_b1:
            b1rt = cpool.tile([P, HID], F32)
            nc.sync.dma_start(b1rt[:], b1r_d.ap()[:])
        if use_b2:
            b2rt = cpool.tile([1, HID], BF16)
            nc.sync.dma_start(b2rt[:], b2r_d.ap()[:])
            m2rt = cpool.tile([1, NBP], BF16)
            nc.sync.dma_start(m2rt[:], m2r_d.ap()[:])
        if use_b4:
            b4rt = cpool.tile([1, FN], BF16)
            nc.sync.dma_start(b4rt[:], b4r_d.ap()[:])
            onesrt = cpool.tile([1, P], BF16)
            nc.sync.dma_start(onesrt[:], onesr_d.ap()[:])
        z512 = cpool.tile([1, 512], BF16)
        nc.vector.memset(z512[:], 0.0)

        # ---------------- Phase E: h1 rows ----------------
        with ExitStack() as ectx:
            inp = ectx.enter_context(tc.tile_pool(name="inT", bufs=2))
            h1p = ectx.enter_context(tc.tile_pool(name="h1r", bufs=4))
            tbp = (ectx.enter_context(tc.tile_pool(name="tb", bufs=2))
                   if use_b1 else None)
            mmp = ectx.enter_context(
                tc.tile_pool(name="mmE", bufs=2, space="PSUM"))

            def load_in(sc):
                t = inp.tile([P, 5, 512], BF16, name=f"in_{sc}", tag="inT")
                nc.sync.dma_start(t[:], inT_d.ap()[:, :, sc * 512:(sc + 1) * 512])
                return t

            in_cur = load_in(0)
            for sc in range(SC):
                in_next = load_in(sc + 1) if sc + 1 < SC else None
                # h1 rows [e, hid1]: stationary inT chunk slices, moving W1
                for c in range(4):
                    ci = sc * 4 + c
                    h1t = h1p.tile([P, HID], BF16, name=f"h1_{sc}_{c}",
                                   tag="h1r")
                    for g in range(3):
                        lo = g * 512
                        hi = min(lo + 512, HID)
                        ps = mmp.tile([P, hi - lo], F32)
                        for k in range(5):
                            nc.tensor.matmul(
                                ps[:], in_cur[:, k, c * P:(c + 1) * P],
                                W1t[:, k, lo:hi], start=(k == 0), stop=(k == 4))
                        if use_b1:
                            tb = tbp.tile([P, hi - lo], F32)
                            nc.vector.tensor_tensor(
                                tb[:], ps[:], b1rt[:, lo:hi],
                                op=mybir.AluOpType.add)
                            nc.scalar.activation(
                                h1t[:, lo:hi], tb[:], RELU,
                                scale=invct[:, ci:ci + 1])
                        else:
                            # relu(psum)*1/deg(dest); scale>0 commutes w/relu
                            nc.scalar.activation(
                                h1t[:, lo:hi], ps[:], RELU,
                                scale=invct[:, ci:ci + 1])
                    r0 = ci * P
                    nc.sync.dma_start(h1_d.ap()[r0:r0 + P, :], h1t[:])
                in_cur = in_next

        # ---------- Phases S+N: scatter-mean, W2, node MLP ----------
        with ExitStack() as sctx:
            h1gp = sctx.enter_context(tc.tile_pool(name="h1g", bufs=3 * KB))
            Sp = sctx.enter_context(tc.tile_pool(name="Smat", bufs=3 * KB))
            agHp = sctx.enter_context(tc.tile_pool(name="agH", bufs=2))
            aggp = sctx.enter_context(tc.tile_pool(name="aggT", bufs=2))
            xsp = sctx.enter_context(tc.tile_pool(name="xs", bufs=2))
            h3p = sctx.enter_context(tc.tile_pool(name="h3T", bufs=2))
            ogp = sctx.enter_context(tc.tile_pool(name="og", bufs=4))
            spp = sctx.enter_context(
                tc.tile_pool(name="spp", bufs=2, space="PSUM"))
            mmp2 = sctx.enter_context(
                tc.tile_pool(name="mmN", bufs=2, space="PSUM"))

            pend_gs = {}

            def gather_S(b):
                lst = []
                for k in range(KB):
                    c = b * KB + k
                    h1g = h1gp.tile([P, HID], BF16, name=f"h1g_{b}_{k}",
                                    tag="h1g")
                    nc.gpsimd.indirect_dma_start(
                        out=h1g[:], out_offset=None, in_=h1_d.ap()[:],
                        in_offset=bass.IndirectOffsetOnAxis(
                            ap=gidt[:, c:c + 1], axis=0),
                        bounds_check=EP - 1, oob_is_err=False)
                    St = Sp.tile([P, P], BF16, name=f"S_{b}_{k}", tag="S")
                    nc.vector.tensor_tensor(
                        St[:], colbt[:, c:c + 1].to_broadcast([P, P]),
                        iotat[:], op=mybir.AluOpType.is_equal)
                    lst.append((h1g, St))
                pend_gs[b] = lst

            gather_S(0)
            gather_S(1)

            def load_xst(s):
                t = xsp.tile([P, 4, 512], BF16, name=f"xst_{s}", tag="xst")
                nc.sync.dma_start(
                    t[:], xsT_d.ap()[:, :, s * 512:(s + 1) * 512])
                return t

            xst_cur = load_xst(0)
            for s in range(NSB):
                agHt = agHp.tile([P, 10, 512], BF16)
                for bb in range(4):
                    b = s * 4 + bb
                    if b + 2 < NB:
                        gather_S(b + 2)
                    gs = pend_gs.pop(b)
                    # 10 sub-bank groups: zero via bank-covering K=1
                    # matmuls, then accumulate with start=False only.
                    sp = spp.tile([P, 10 * P], F32)
                    for z0 in range(0, 10 * P, 512):
                        zw = min(512, 10 * P - z0)
                        nc.tensor.matmul(
                            sp[:, z0:z0 + zw], z512[0:1, 0:P],
                            z512[0:1, 0:zw], start=True, stop=False)
                    for k, (h1g, St) in enumerate(gs):
                        for hs in range(10):
                            nc.tensor.matmul(
                                sp[:, hs * P:(hs + 1) * P],
                                h1g[:, hs * P:(hs + 1) * P],
                                St[:], start=False, stop=(k == KB - 1))
                    for hs in range(10):
                        nc.scalar.activation(
                            agHt[:, hs, bb * P:(bb + 1) * P],
                            sp[:, hs * P:(hs + 1) * P], COPY)

                xst = xst_cur
                xst_cur = load_xst(s + 1) if s + 1 < NSB else None
                # aggT [hid2, n] = W2^T @ aggH1T (+ b2*[deg>0])
                aggTt = aggp.tile([P, 10, 512], BF16)
                for of in range(10):
                    ps = mmp2.tile([P, 512], F32)
                    for k in range(10):
                        nc.tensor.matmul(
                            ps[:], W2t[:, k, of * P:(of + 1) * P],
                            agHt[:, k, :], start=(k == 0),
                            stop=(k == 9 and not use_b2))
                    if use_b2:
                        nc.tensor.matmul(
                            ps[:], b2rt[0:1, of * P:(of + 1) * P],
                            m2rt[0:1, s * 512:(s + 1) * 512],
                            start=False, stop=True)
                    nc.scalar.activation(aggTt[:, of, :], ps[:], COPY)
                # h3T [of, n]: stationary W3 slices, moving xsT/aggT
                h3Tt = h3p.tile([P, 5, 512], BF16)
                for of in range(5):
                    ps = mmp2.tile([P, 512], F32)
                    for k in range(4):
                        nc.tensor.matmul(
                            ps[:], W3t[:, k, of * P:(of + 1) * P],
                            xst[:, k, :], start=(k == 0), stop=False)
                    for k in range(10):
                        nc.tensor.matmul(
                            ps[:], W3t[:, 4 + k, of * P:(of + 1) * P],
                            aggTt[:, k, :], start=False, stop=(k == 9))
                    nc.scalar.activation(h3Tt[:, of, :], ps[:], RELU,
                                         bias=b3t[:, of:of + 1])
                # out [n, feat] rows: stationary h3T slices, moving W4
                for c in range(4):
                    ps = mmp2.tile([P, FN], F32)
                    for k in range(5):
                        nc.tensor.matmul(
                            ps[:], h3Tt[:, k, c * P:(c + 1) * P],
                            W4t[:, k, :], start=(k == 0),
                            stop=(k == 4 and not use_b4))
                    if use_b4:
                        nc.tensor.matmul(
                            ps[:], onesrt[0:1, :], b4rt[0:1, :],
                            start=False, stop=True)
                    og = ogp.tile([P, FN], F32, name=f"og_{s}_{c}", tag="og")
                    nc.scalar.activation(og[:], ps[:], COPY)
                    r0 = s * 512 + c * P
                    nc.sync.dma_start(out_d.ap()[r0:r0 + P, :], og[:])

    nc.compile()
    return nc


def _prepare(x8, row, col, ea8):
    """Host-side sharding: sort edges by destination, split nodes into 8
    block-aligned edge-balanced shards, build per-core arrays (bf16)."""
    N = x8.shape[0]
    E = ea8.shape[0]
    order = np.argsort(col, kind="stable")
    scol = col[order]
    srow = row[order]
    NBLK = (N + P - 1) // P
    NTOT = NBLK * P

    bounds = [0]
    for p in range(1, NCORES):
        if E > 0:
            t = int(scol[min((p * E) // NCORES, E - 1)])
        else:
            t = (p * NTOT) // NCORES
        b = int(round(t / P)) * P
        b = max(b, bounds[-1] + P)
        b = min(b, NTOT - P * (NCORES - p))
        bounds.append(b)
    bounds.append(NTOT)
    for p in range(1, NCORES + 1):
        assert bounds[p] > bounds[p - 1], f"degenerate shard bounds {bounds}"

    e_split = np.searchsorted(scol, bounds)
    Ec = np.diff(e_split)
    EC = max(4, math.ceil(int(Ec.max()) / P))
    EC = ((EC + 3) // 4) * 4
    EP = EC * P
    nblk = [(bounds[p + 1] - bounds[p]) // P for p in range(NCORES)]
    NB = max(4, ((max(nblk) + 3) // 4) * 4)
    NBP = NB * P
    blkdeg = np.bincount(scol // P, minlength=NBLK)
    KB = max(1, math.ceil(int(blkdeg.max()) / P))

    deg = np.bincount(scol, minlength=NTOT + NBP).astype(np.float32)
    inve_all = 1.0 / np.maximum(deg[scol], 1.0)  # per sorted edge

    xpadT = np.zeros((FN, NTOT + NBP), NPBF)
    xpadT[:, :N] = x8.T

    cores = []
    for p in range(NCORES):
        s, e = int(e_split[p]), int(e_split[p + 1])
        n0 = bounds[p]
        ne = e - s
        # gathered+transposed edge-MLP input [ki, ko, e]
        feat = np.zeros((EP, IN1), NPBF)
        feat[:ne, :FN] = x8[srow[s:e]]
        feat[:ne, FN:] = ea8[order[s:e]]
        inT = np.ascontiguousarray(
            feat.T.reshape(5, P, EP).transpose(1, 0, 2))
        # per-edge 1/deg(dest) in [ki, chunk] layout
        ive = np.zeros(EP, np.float32)
        ive[:ne] = inve_all[s:e]
        invce = np.ascontiguousarray(ive.reshape(EC, P).T)
        # scatter schedule
        lcol = (scol[s:e] - n0).astype(np.int64)
        bstart = np.searchsorted(lcol, np.arange(NB + 1) * P)
        gid = np.full((NB, KB, P), 1 << 30, np.int32)
        gid.reshape(NB * KB, P)[:3 * KB] = 0  # first tiles: finite data
        colb = np.full((NB, KB, P), -1.0, np.float32)
        for b in range(NB):
            sb, eb = int(bstart[b]), int(bstart[b + 1])
            cnt = eb - sb
            assert cnt <= KB * P
            gid[b].reshape(-1)[:cnt] = np.arange(sb, eb, dtype=np.int32)
            colb[b].reshape(-1)[:cnt] = (lcol[sb:eb] - b * P)
        gid_t = np.ascontiguousarray(gid.reshape(NB * KB, P).T)
        colb_t = np.ascontiguousarray(colb.reshape(NB * KB, P).T)
        xsT = np.ascontiguousarray(
            xpadT[:, n0:n0 + NBP].reshape(4, P, NBP).transpose(1, 0, 2))
        ndeg = deg[n0:n0 + NBP]
        cores.append(dict(inT=inT, invce=invce, gid=gid_t, colb=colb_t,
                          xsT=xsT, ndeg=ndeg))
    return cores, bounds, EC, NB, KB


def _run(inputs, trace=False):
    x = np.asarray(inputs["x"], dtype=np.float32)
    ei = np.asarray(inputs["edge_index"])
    ea = np.asarray(inputs["edge_attr"], dtype=np.float32)
    row = ei[0].astype(np.int64)
    col = ei[1].astype(np.int64)
    x8 = x.astype(NPBF)
    ea8 = ea.astype(NPBF)
    W1 = np.ascontiguousarray(np.asarray(inputs["W1"], np.float32)).astype(NPBF)
    W2 = np.ascontiguousarray(np.asarray(inputs["W2"], np.float32)).astype(NPBF)
    W3 = np.ascontiguousarray(np.asarray(inputs["W3"], np.float32)).astype(NPBF)
    W4 = np.ascontiguousarray(np.asarray(inputs["W4"], np.float32)).astype(NPBF)
    b1 = np.asarray(inputs["b1"], np.float32)
    b2 = np.asarray(inputs["b2"], np.float32)
    b3 = np.asarray(inputs["b3"], np.float32)
    b4 = np.asarray(inputs["b4"], np.float32)
    N = x.shape[0]

    cores, bounds, EC, NB, KB = _prepare(x8, row, col, ea8)
    use_b1 = bool(np.any(b1))
    use_b2 = bool(np.any(b2))
    use_b4 = bool(np.any(b4))

    key = (EC, NB, KB, use_b1, use_b2, use_b4)
    if key not in _prog_cache:
        _prog_cache[key] = _build(EC, NB, KB, use_b1, use_b2, use_b4)
    nc = _prog_cache[key]

    b3t = np.ascontiguousarray(b3.reshape(IN1 // P, P).T)
    iota = np.ascontiguousarray(
        np.broadcast_to(np.arange(P, dtype=np.float32), (P, P)))

    in_maps = []
    for p in range(NCORES):
        c = cores[p]
        m = {
            "inT": c["inT"], "W1": W1, "W2": W2, "W3": W3, "W4": W4,
            "b3": b3t, "gid": c["gid"], "colb": c["colb"],
            "invce": c["invce"], "xsT": c["xsT"], "iota": iota,
        }
        if use_b1:
            m["b1r"] = np.ascontiguousarray(
                np.broadcast_to(b1.reshape(1, HID), (P, HID))).astype(
                    np.float32)
        if use_b2:
            m["b2r"] = np.ascontiguousarray(b2.reshape(1, HID)).astype(NPBF)
            m["m2r"] = (c["ndeg"] > 0).reshape(1, -1).astype(NPBF)
        if use_b4:
            m["b4r"] = np.ascontiguousarray(b4.reshape(1, FN)).astype(NPBF)
            m["onesr"] = np.ones((1, P), NPBF)
        in_maps.append(m)

    res = run_bass_kernel_spmd(nc, in_maps, list(range(NCORES)), trace=trace)

    out = np.empty((N, FN), np.float32)
    for p in range(NCORES):
        n0, n1 = bounds[p], min(bounds[p + 1], N)
        if n1 > n0:
            out[n0:n1] = res.results[p]["out"][:n1 - n0]
    return out, res


def kernel(**inputs) -> np.ndarray:
    out, _ = _run(inputs, trace=False)
    return out


# revision 14
# speedup vs baseline: 1.6905x; 1.0214x over previous
"""Trainium2 Bass kernel for nn_NodeModel (GNN message passing), bf16.

Reference computation:
    h   = relu(concat(x[row], edge_attr) @ W1 + b1) @ W2 + b2     # edge MLP
    agg = scatter_mean(h, col, N)                                  # per-dest mean
    out = relu(concat(x, agg) @ W3 + b3) @ W4 + b4                 # node MLP

Key algebraic restructure: scatter_mean is linear, so W2 commutes with it:
    agg = scatter_mean(relu(h1), col) @ W2 + b2*[deg>0]
which applies the 1280x1280 W2 matmul per NODE (~6.3k rows/core) instead
of per EDGE (~16k rows/core) — a ~2.5x FLOP cut on the largest matmul.

Distribution strategy (8 cores, no collectives):
  - Sort edges by destination node; split destination nodes into 8
    block-aligned, edge-balanced shards.  Each core owns one node shard and
    ALL edges targeting it, so per-node means are complete locally.
  - The gathered-and-transposed edge input concat(x[row], edge_attr)^T is
    prepared host-side per core, so the device runs no transposes at all:
      * h1 rows [edge, 1280] are computed directly (inT chunk slices
        stationary, W1 moving), relu'd and pre-scaled by 1/deg(dest) at
        PSUM drain, staged to DRAM,
      * scatter-mean gathers the rows of each 128-node destination block
        and matmuls h1_slice^T @ onehot(S), accumulating agg^T directly,
      * aggT = W2^T @ aggH1T per superblock (W2 slices stationary),
      * node MLP consumes agg^T and x^T and emits output rows directly.
  - All matmuls run in bfloat16 (1 cycle/row on the PE, like fp32r, but
    half the SBUF/DMA traffic, no fp32r small-free-dim penalty, and no
    transpose/HAM-throttle); accumulation is fp32 in PSUM.  fp8 was
    measured numerically out of reach for the 2e-2 gate.
  - PSUM accumulation groups are kept bank-safe: the 10 per-block scatter
    groups (512B each, sub-bank) are zero-initialized by three K=1
    bank-covering matmuls, and every real matmul accumulates with
    start=False (start_tensor_calc zeroes whole 2KB banks and would
    corrupt bank neighbours).
"""

import math
import sys
from contextlib import ExitStack

sys.path.insert(0, "/opt/trn_rl_repo")

import numpy as np
import ml_dtypes

import concourse.bass as bass
import concourse.tile as tile
from concourse import bacc, mybir
from concourse.bass_utils import run_bass_kernel_spmd

NCORES = 8
P = 128
FN = 512    # node feature dim
FE = 128    # edge feature dim
HID = 1280  # edge-MLP hidden/output dim
IN1 = FN + FE          # 640
IN2 = FN + HID         # 1792
BF16 = mybir.dt.bfloat16
F32 = mybir.dt.float32
I32 = mybir.dt.int32
RELU = mybir.ActivationFunctionType.Relu
COPY = mybir.ActivationFunctionType.Copy
NPBF = ml_dtypes.bfloat16

_prog_cache = {}


def _build(EC, NB, KB, use_b1, use_b2, use_b4):
    """Build the SPMD program for one core.

    EC: edge chunks (128 edges each) per core, multiple of 4.
    NB: node blocks (128 nodes each) per core, multiple of 4.
    KB: max edge chunks per node block (scatter schedule width).
    """
    EP = EC * P
    NBP = NB * P
    SC = EC // 4   # superchunks of 512 edges
    NSB = NB // 4  # superblocks of 512 nodes

    nc = bacc.Bacc("TRN2", target_bir_lowering=False, debug=False,
                   num_devices=NCORES)

    inT_d = nc.dram_tensor("inT", [P, 5, EP], BF16, kind="ExternalInput")
    W1_d = nc.dram_tensor("W1", [IN1, HID], BF16, kind="ExternalInput")
    W2_d = nc.dram_tensor("W2", [HID, HID], BF16, kind="ExternalInput")
    W3_d = nc.dram_tensor("W3", [IN2, IN1], BF16, kind="ExternalInput")
    W4_d = nc.dram_tensor("W4", [IN1, FN], BF16, kind="ExternalInput")
    b3_d = nc.dram_tensor("b3", [P, IN1 // P], F32, kind="ExternalInput")
    gid_d = nc.dram_tensor("gid", [P, NB * KB], I32, kind="ExternalInput")
    colb_d = nc.dram_tensor("colb", [P, NB * KB], F32, kind="ExternalInput")
    invce_d = nc.dram_tensor("invce", [P, EC], F32, kind="ExternalInput")
    xsT_d = nc.dram_tensor("xsT", [P, 4, NBP], BF16, kind="ExternalInput")
    iota_d = nc.dram_tensor("iota", [P, P], F32, kind="ExternalInput")
    if use_b1:
        b1r_d = nc.dram_tensor("b1r", [P, HID], F32, kind="ExternalInput")
    if use_b2:
        b2r_d = nc.dram_tensor("b2r", [1, HID], BF16, kind="ExternalInput")
        m2r_d = nc.dram_tensor("m2r", [1, NBP], BF16, kind="ExternalInput")
    if use_b4:
        b4r_d = nc.dram_tensor("b4r", [1, FN], BF16, kind="ExternalInput")
        onesr_d = nc.dram_tensor("onesr", [1, P], BF16, kind="ExternalInput")
    out_d = nc.dram_tensor("out", [NBP, FN], F32, kind="ExternalOutput")
    h1_d = nc.dram_tensor("h1buf", [EP, HID], BF16)  # internal staging

    with tile.TileContext(nc) as tc, ExitStack() as ctx:
        cpool = ctx.enter_context(tc.tile_pool(name="const", bufs=1))

        iotat = cpool.tile([P, P], F32)
        nc.sync.dma_start(iotat[:], iota_d.ap()[:])
        b3t = cpool.tile([P, IN1 // P], F32)
        nc.sync.dma_start(b3t[:], b3_d.ap()[:])
        gidt = cpool.tile([P, NB * KB], I32)
        nc.sync.dma_start(gidt[:], gid_d.ap()[:])
        colbt = cpool.tile([P, NB * KB], F32)
        nc.sync.dma_start(colbt[:], colb_d.ap()[:])
        invct = cpool.tile([P, EC], F32)
        nc.sync.dma_start(invct[:], invce_d.ap()[:])
        W1t = cpool.tile([P, 5, HID], BF16)
        W1r = W1_d.ap().rearrange("(ko ki) m -> ki ko m", ki=P)
        for k in range(5):
            nc.sync.dma_start(W1t[:, k, :], W1r[:, k, :])
        W2t = cpool.tile([P, 10, HID], BF16)
        W2r = W2_d.ap().rearrange("(ko ki) m -> ki ko m", ki=P)
        for k in range(10):
            nc.sync.dma_start(W2t[:, k, :], W2r[:, k, :])
        W3t = cpool.tile([P, 14, IN1], BF16)
        nc.sync.dma_start(
            W3t[:], W3_d.ap().rearrange("(ko ki) m -> ki ko m", ki=P))
        W4t = cpool.tile([P, 5, FN], BF16)
        nc.sync.dma_start(
            W4t[:], W4_d.ap().rearrange("(ko ki) m -> ki ko m", ki=P))
        if use_b1:
            b1rt = cpool.tile([P, HID], F32)
            nc.sync.dma_start(b1rt[:], b1r_d.ap()[:])
        if use_b2:
            b2rt = cpool.tile([1, HID], BF16)
            nc.sync.dma_start(b2rt[:], b2r_d.ap()[:])
            m2rt = cpool.tile([1, NBP], BF16)
            nc.sync.dma_start(m2rt[:], m2r_d.ap()[:])
        if use_b4:
            b4rt = cpool.tile([1, FN], BF16)
            nc.sync.dma_start(b4rt[:], b4r_d.ap()[:])
            onesrt = cpool.tile([1, P], BF16)
            nc.sync.dma_start(onesrt[:], onesr_d.ap()[:])
        z512 = cpool.tile([1, 512], BF16)
        nc.vector.memset(z512[:], 0.0)

        # ---------------- Phase E: h1 rows ----------------
        with ExitStack() as ectx:
            inp = ectx.enter_context(tc.tile_pool(name="inT", bufs=2))
            h1p = ectx.enter_context(tc.tile_pool(name="h1r", bufs=4))
            tbp = (ectx.enter_context(tc.tile_pool(name="tb", bufs=2))
                   if use_b1 else None)
            mmp = ectx.enter_context(
                tc.tile_pool(name="mmE", bufs=2, space="PSUM"))

            def load_in(sc):
                t = inp.tile([P, 5, 512], BF16, name=f"in_{sc}", tag="inT")
                nc.sync.dma_start(t[:], inT_d.ap()[:, :, sc * 512:(sc + 1) * 512])
                return t

            in_cur = load_in(0)
            for sc in range(SC):
                in_next = load_in(sc + 1) if sc + 1 < SC else None
                # h1 rows [e, hid1]: stationary inT chunk slices, moving W1
                for c in range(4):
                    ci = sc * 4 + c
                    h1t = h1p.tile([P, HID], BF16, name=f"h1_{sc}_{c}",
                                   tag="h1r")
                    for g in range(3):
                        lo = g * 512
                        hi = min(lo + 512, HID)
                        ps = mmp.tile([P, hi - lo], F32)
                        for k in range(5):
                            nc.tensor.matmul(
                                ps[:], in_cur[:, k, c * P:(c + 1) * P],
                                W1t[:, k, lo:hi], start=(k == 0), stop=(k == 4))
                        if use_b1:
                            tb = tbp.tile([P, hi - lo], F32)
                            nc.vector.tensor_tensor(
                                tb[:], ps[:], b1rt[:, lo:hi],
                                op=mybir.AluOpType.add)
                            nc.scalar.activation(
                                h1t[:, lo:hi], tb[:], RELU,
                                scale=invct[:, ci:ci + 1])
                        else:
                            # max(psum*invc, 0) = relu(psum)/deg on DVE
                            nc.vector.tensor_scalar(
                                h1t[:, lo:hi], ps[:],
                                invct[:, ci:ci + 1], 0.0,
                                op0=mybir.AluOpType.mult,
                                op1=mybir.AluOpType.max)
                    r0 = ci * P
                    nc.sync.dma_start(h1_d.ap()[r0:r0 + P, :], h1t[:])
                in_cur = in_next

        # ---------- Phases S+N: scatter-mean, W2, node MLP ----------
        with ExitStack() as sctx:
            h1gp = sctx.enter_context(tc.tile_pool(name="h1g", bufs=3 * KB))
            Sp = sctx.enter_context(tc.tile_pool(name="Smat", bufs=3 * KB))
            agHp = sctx.enter_context(tc.tile_pool(name="agH", bufs=2))
            aggp = sctx.enter_context(tc.tile_pool(name="aggT", bufs=2))
            xsp = sctx.enter_context(tc.tile_pool(name="xs", bufs=2))
            h3p = sctx.enter_context(tc.tile_pool(name="h3T", bufs=2))
            ogp = sctx.enter_context(tc.tile_pool(name="og", bufs=4))
            spp = sctx.enter_context(
                tc.tile_pool(name="spp", bufs=2, space="PSUM"))
            mmp2 = sctx.enter_context(
                tc.tile_pool(name="mmN", bufs=2, space="PSUM"))

            pend_gs = {}

            def gather_S(b):
                lst = []
                for k in range(KB):
                    c = b * KB + k
                    h1g = h1gp.tile([P, HID], BF16, name=f"h1g_{b}_{k}",
                                    tag="h1g")
                    nc.gpsimd.indirect_dma_start(
                        out=h1g[:], out_offset=None, in_=h1_d.ap()[:],
                        in_offset=bass.IndirectOffsetOnAxis(
                            ap=gidt[:, c:c + 1], axis=0),
                        bounds_check=EP - 1, oob_is_err=False)
                    St = Sp.tile([P, P], BF16, name=f"S_{b}_{k}", tag="S")
                    nc.vector.tensor_tensor(
                        St[:], colbt[:, c:c + 1].to_broadcast([P, P]),
                        iotat[:], op=mybir.AluOpType.is_equal)
                    lst.append((h1g, St))
                pend_gs[b] = lst

            gather_S(0)
            gather_S(1)

            def load_xst(s):
                t = xsp.tile([P, 4, 512], BF16, name=f"xst_{s}", tag="xst")
                nc.sync.dma_start(
                    t[:], xsT_d.ap()[:, :, s * 512:(s + 1) * 512])
                return t

            xst_cur = load_xst(0)
            for s in range(NSB):
                agHt = agHp.tile([P, 10, 512], BF16)
                for bb in range(4):
                    b = s * 4 + bb
                    if b + 2 < NB:
                        gather_S(b + 2)
                    gs = pend_gs.pop(b)
                    # 10 sub-bank accumulation groups: zero the tile on the
                    # DVE, then accumulate every matmul with start=False
                    # (start_tensor_calc zeroes whole 2KB banks and would
                    # corrupt bank neighbours).
                    sp = spp.tile([P, 10 * P], F32)
                    nc.vector.memset(sp[:], 0.0)
                    for k, (h1g, St) in enumerate(gs):
                        for hs in range(10):
                            nc.tensor.matmul(
                                sp[:, hs * P:(hs + 1) * P],
                                h1g[:, hs * P:(hs + 1) * P],
                                St[:], start=False, stop=(k == KB - 1))
                    for hs in range(10):
                        nc.vector.tensor_copy(
                            agHt[:, hs, bb * P:(bb + 1) * P],
                            sp[:, hs * P:(hs + 1) * P])

                xst = xst_cur
                xst_cur = load_xst(s + 1) if s + 1 < NSB else None
                # aggT [hid2, n] = W2^T @ aggH1T (+ b2*[deg>0])
                aggTt = aggp.tile([P, 10, 512], BF16)
                for of in range(10):
                    ps = mmp2.tile([P, 512], F32)
                    for k in range(10):
                        nc.tensor.matmul(
                            ps[:], W2t[:, k, of * P:(of + 1) * P],
                            agHt[:, k, :], start=(k == 0),
                            stop=(k == 9 and not use_b2))
                    if use_b2:
                        nc.tensor.matmul(
                            ps[:], b2rt[0:1, of * P:(of + 1) * P],
                            m2rt[0:1, s * 512:(s + 1) * 512],
                            start=False, stop=True)
                    nc.vector.tensor_copy(aggTt[:, of, :], ps[:])
                # h3T [of, n]: stationary W3 slices, moving xsT/aggT
                h3Tt = h3p.tile([P, 5, 512], BF16)
                for of in range(5):
                    ps = mmp2.tile([P, 512], F32)
                    for k in range(4):
                        nc.tensor.matmul(
                            ps[:], W3t[:, k, of * P:(of + 1) * P],
                            xst[:, k, :], start=(k == 0), stop=False)
                    for k in range(10):
                        nc.tensor.matmul(
                            ps[:], W3t[:, 4 + k, of * P:(of + 1) * P],
                            aggTt[:, k, :], start=False, stop=(k == 9))
                    nc.scalar.activation(h3Tt[:, of, :], ps[:], RELU,
                                         bias=b3t[:, of:of + 1])
                # out [n, feat] rows: stationary h3T slices, moving W4
                for c in range(4):
                    ps = mmp2.tile([P, FN], F32)
                    for k in range(5):
                        nc.tensor.matmul(
                            ps[:], h3Tt[:, k, c * P:(c + 1) * P],
                            W4t[:, k, :], start=(k == 0),
                            stop=(k == 4 and not use_b4))
                    if use_b4:
                        nc.tensor.matmul(
                            ps[:], onesrt[0:1, :], b4rt[0:1, :],
                            start=False, stop=True)
                    og = ogp.tile([P, FN], F32, name=f"og_{s}_{c}", tag="og")
                    nc.vector.tensor_copy(og[:], ps[:])
                    r0 = s * 512 + c * P
                    nc.sync.dma_start(out_d.ap()[r0:r0 + P, :], og[:])

    nc.compile()
    return nc


def _prepare(x8, row, col, ea8):
    """Host-side sharding: sort edges by destination, split nodes into 8
    block-aligned edge-balanced shards, build per-core arrays (bf16)."""
    N = x8.shape[0]
    E = ea8.shape[0]
    order = np.argsort(col, kind="stable")
    scol = col[order]
    srow = row[order]
    NBLK = (N + P - 1) // P
    NTOT = NBLK * P

    bounds = [0]
    for p in range(1, NCORES):
        if E > 0:
            t = int(scol[min((p * E) // NCORES, E - 1)])
        else:
            t = (p * NTOT) // NCORES
        b = int(round(t / P)) * P
        b = max(b, bounds[-1] + P)
        b = min(b, NTOT - P * (NCORES - p))
        bounds.append(b)
    bounds.append(NTOT)
    for p in range(1, NCORES + 1):
        assert bounds[p] > bounds[p - 1], f"degenerate shard bounds {bounds}"

    e_split = np.searchsorted(scol, bounds)
    Ec = np.diff(e_split)
    EC = max(4, math.ceil(int(Ec.max()) / P))
    EC = ((EC + 3) // 4) * 4
    EP = EC * P
    nblk = [(bounds[p + 1] - bounds[p]) // P for p in range(NCORES)]
    NB = max(4, ((max(nblk) + 3) // 4) * 4)
    NBP = NB * P
    blkdeg = np.bincount(scol // P, minlength=NBLK)
    KB = max(1, math.ceil(int(blkdeg.max()) / P))

    deg = np.bincount(scol, minlength=NTOT + NBP).astype(np.float32)
    inve_all = 1.0 / np.maximum(deg[scol], 1.0)  # per sorted edge

    xpadT = np.zeros((FN, NTOT + NBP), NPBF)
    xpadT[:, :N] = x8.T

    cores = []
    for p in range(NCORES):
        s, e = int(e_split[p]), int(e_split[p + 1])
        n0 = bounds[p]
        ne = e - s
        # gathered+transposed edge-MLP input [ki, ko, e]
        feat = np.zeros((EP, IN1), NPBF)
        feat[:ne, :FN] = x8[srow[s:e]]
        feat[:ne, FN:] = ea8[order[s:e]]
        inT = np.ascontiguousarray(
            feat.T.reshape(5, P, EP).transpose(1, 0, 2))
        # per-edge 1/deg(dest) in [ki, chunk] layout
        ive = np.zeros(EP, np.float32)
        ive[:ne] = inve_all[s:e]
        invce = np.ascontiguousarray(ive.reshape(EC, P).T)
        # scatter schedule
        lcol = (scol[s:e] - n0).astype(np.int64)
        bstart = np.searchsorted(lcol, np.arange(NB + 1) * P)
        gid = np.full((NB, KB, P), 1 << 30, np.int32)
        gid.reshape(NB * KB, P)[:3 * KB] = 0  # first tiles: finite data
        colb = np.full((NB, KB, P), -1.0, np.float32)
        for b in range(NB):
            sb, eb = int(bstart[b]), int(bstart[b + 1])
            cnt = eb - sb
            assert cnt <= KB * P
            gid[b].reshape(-1)[:cnt] = np.arange(sb, eb, dtype=np.int32)
            colb[b].reshape(-1)[:cnt] = (lcol[sb:eb] - b * P)
        gid_t = np.ascontiguousarray(gid.reshape(NB * KB, P).T)
        colb_t = np.ascontiguousarray(colb.reshape(NB * KB, P).T)
        xsT = np.ascontiguousarray(
            xpadT[:, n0:n0 + NBP].reshape(4, P, NBP).transpose(1, 0, 2))
        ndeg = deg[n0:n0 + NBP]
        cores.append(dict(inT=inT, invce=invce, gid=gid_t, colb=colb_t,
                          xsT=xsT, ndeg=ndeg))
    return cores, bounds, EC, NB, KB


def _run(inputs, trace=False):
    x = np.asarray(inputs["x"], dtype=np.float32)
    ei = np.asarray(inputs["edge_index"])
    ea = np.asarray(inputs["edge_attr"], dtype=np.float32)
    row = ei[0].astype(np.int64)
    col = ei[1].astype(np.int64)
    x8 = x.astype(NPBF)
    ea8 = ea.astype(NPBF)
    W1 = np.ascontiguousarray(np.asarray(inputs["W1"], np.float32)).astype(NPBF)
    W2 = np.ascontiguousarray(np.asarray(inputs["W2"], np.float32)).astype(NPBF)
    W3 = np.ascontiguousarray(np.asarray(inputs["W3"], np.float32)).astype(NPBF)
    W4 = np.ascontiguousarray(np.asarray(inputs["W4"], np.float32)).astype(NPBF)
    b1 = np.asarray(inputs["b1"], np.float32)
    b2 = np.asarray(inputs["b2"], np.float32)
    b3 = np.asarray(inputs["b3"], np.float32)
    b4 = np.asarray(inputs["b4"], np.float32)
    N = x.shape[0]

    cores, bounds, EC, NB, KB = _prepare(x8, row, col, ea8)
    use_b1 = bool(np.any(b1))
    use_b2 = bool(np.any(b2))
    use_b4 = bool(np.any(b4))

    key = (EC, NB, KB, use_b1, use_b2, use_b4)
    if key not in _prog_cache:
        _prog_cache[key] = _build(EC, NB, KB, use_b1, use_b2, use_b4)
    nc = _prog_cache[key]

    b3t = np.ascontiguousarray(b3.reshape(IN1 // P, P).T)
    iota = np.ascontiguousarray(
        np.broadcast_to(np.arange(P, dtype=np.float32), (P, P)))

    in_maps = []
    for p in range(NCORES):
        c = cores[p]
        m = {
            "inT": c["inT"], "W1": W1, "W2": W2, "W3": W3, "W4": W4,
            "b3": b3t, "gid": c["gid"], "colb": c["colb"],
            "invce": c["invce"], "xsT": c["xsT"], "iota": iota,
        }
        if use_b1:
            m["b1r"] = np.ascontiguousarray(
                np.broadcast_to(b1.reshape(1, HID), (P, HID))).astype(
                    np.float32)
        if use_b2:
            m["b2r"] = np.ascontiguousarray(b2.reshape(1, HID)).astype(NPBF)
            m["m2r"] = (c["ndeg"] > 0).reshape(1, -1).astype(NPBF)
        if use_b4:
            m["b4r"] = np.ascontiguousarray(b4.reshape(1, FN)).astype(NPBF)
            m["onesr"] = np.ones((1, P), NPBF)
        in_maps.append(m)

    res = run_bass_kernel_spmd(nc, in_maps, list(range(NCORES)), trace=trace)

    out = np.empty((N, FN), np.float32)
    for p in range(NCORES):
        n0, n1 = bounds[p], min(bounds[p + 1], N)
        if n1 > n0:
            out[n0:n1] = res.results[p]["out"][:n1 - n0]
    return out, res


def kernel(**inputs) -> np.ndarray:
    out, _ = _run(inputs, trace=False)
    return out
